# revision 7
# baseline (speedup 1.0000x reference)
"""Causal GQA attention (prefill) on 8 TRN2 NeuronCores — v2.

Problem: B=2, S=2048, H=32 query heads, Hk=8 kv heads, D=128, f32 I/O.
Sharding: tensor-parallel over heads -- core c gets query heads [4c, 4c+4)
and kv head c. Attention is fully independent per head: no collectives.

v2 vs baseline (222us): the baseline was ScalarE-bound — 320 small exp
ACTIVATEs paid a ~352-cycle fixed cost each (~180us ScalarE busy).
  - Q^T/K^T/V+ones are prepared on the HOST (numpy transpose + bf16
    cast): no PE transposes, no DVE casts/copies, minimal input DMA.
  - Scores S^T[k,q] land in rotating 3-bank f32 PSUM regions holding up
    to 3 key-blocks (or the whole packed ragged diagonal superblock);
    exp runs as ~13 big multi-bank ACTIVATEs per instance instead of 40.
  - PV keeps P^T-stationary matmuls streaming V|ones (free softmax
    denominator in psum column 128); the 4 per-superblock accumulators
    share 2 PSUM banks, zero-initialized by a cheap all-zeros matmul so
    accumulate flags never depend on bank-clear semantics.
  - Emission runs a 2-group-deep software pipeline (PE does QK(g+1),
    QK(g+2) while ScalarE exps group g) and normalization is interleaved
    per 128-query-block so accumulator banks recycle without PE stalls.
"""

import ml_dtypes
import numpy as np

import concourse.bass as bass
import concourse.tile as tile
from concourse import bacc, mybir
from concourse.bass import ts
from concourse.bass_utils import run_bass_kernel_spmd
from concourse.masks import make_upper_triangular

B = 2
S = 2048
H = 32
HK = 8
D = 128
NCORES = 8
GH = H // NCORES  # query heads per core (= group size here)
SCALE = 0.08838834764831845  # 1/sqrt(128)

F32 = mybir.dt.float32
BF16 = mybir.dt.bfloat16
BF16_NP = ml_dtypes.bfloat16

NQB = S // 128  # 16 key blocks of 128
NSB = 4  # query superblocks of 512
SCW = 1536  # score region cols (3 PSUM banks of 512 f32)

# Ragged diagonal superblock: seg j covers queries [128j, 512) of the
# superblock. Column packing avoids crossing 512-col bank boundaries:
# [j0 512 | j1 384 | j3 128 | j2 256] -> contiguous cols [0, 1280).
DIAG_COFF = {0: 0, 1: 512, 2: 1024, 3: 896}


def groups_for_sq(sq):
    """Each group: (segs, is_diag); seg = (ki, col_off, width, q_off)."""
    out = []
    clean = list(range(4 * sq))
    for a in range(0, len(clean), 3):
        chunk = clean[a : a + 3]
        out.append(([(ki, 512 * i, 512, 0) for i, ki in enumerate(chunk)], False))
    d = 4 * sq
    out.append((
        [(d + j, DIAG_COFF[j], 512 - 128 * j, 128 * j) for j in range(4)],
        True,
    ))
    return out


def build_nc() -> bass.Bass:
    nc = bacc.Bacc(
        "TRN2", target_bir_lowering=False, debug=False, num_devices=NCORES
    )
    qt_d = nc.declare_dram_parameter("qt", [128, B, GH, S], BF16, isOutput=False)
    kt_d = nc.declare_dram_parameter("kt", [128, B, S], BF16, isOutput=False)
    vx_d = nc.declare_dram_parameter("vx", [128, B, NQB, 129], BF16, isOutput=False)
    o_d = nc.declare_dram_parameter("out", [B, S, GH, D], F32, isOutput=True)

    with tile.TileContext(nc) as tc:
        with (
            tc.tile_pool(name="consts", bufs=1) as consts,
            tc.tile_pool(name="pt", bufs=6) as pt_pool,
            tc.tile_pool(name="oall", bufs=2) as oall_pool,
            tc.tile_pool(name="small", bufs=8) as small_pool,
            tc.tile_pool(name="psum", bufs=1, space="PSUM") as psum_pool,
        ):
            # mask[k, q] = 1 where q >= k (keep), 0 where q < k.
            mask = consts.tile([128, 128], BF16)
            make_upper_triangular(nc, mask, val=1.0, diag=True)
            zeros = consts.tile([128, 128], BF16)
            nc.vector.memset(zeros, 0.0)

            qt = consts.tile([128, B, GH, S], BF16)
            kt = consts.tile([128, B, S], BF16)
            vx = consts.tile([128, B, NQB, 129], BF16)
            # priority order: everything instance 0 needs comes first
            nc.sync.dma_start(out=kt[:, 0, :], in_=kt_d[:, 0, :])
            nc.sync.dma_start(out=vx[:, 0, :, :], in_=vx_d[:, 0, :, :])
            nc.sync.dma_start(out=qt[:, 0, 0, :], in_=qt_d[:, 0, 0, :])
            for g in range(1, GH):
                nc.sync.dma_start(out=qt[:, 0, g, :], in_=qt_d[:, 0, g, :])
            nc.sync.dma_start(out=kt[:, 1, :], in_=kt_d[:, 1, :])
            nc.sync.dma_start(out=vx[:, 1, :, :], in_=vx_d[:, 1, :, :])
            for g in range(GH):
                nc.sync.dma_start(out=qt[:, 1, g, :], in_=qt_d[:, 1, g, :])

            # Persistent per-sq PV accumulators. po_a packs j=0..2 in one
            # bank (130-col stride keeps 8B alignment), po_b holds j=3.
            po_a = psum_pool.tile([128, 3, 130], F32, name="po_a")
            po_b = psum_pool.tile([128, 129], F32, name="po_b")

            def po_ap(j):
                return po_a[:, j, 0:129] if j < 3 else po_b

            def emit_qk(item):
                (segs, _), b, g, sq = item[0], item[1], item[2], item[3]
                sc = psum_pool.tile([128, SCW], F32, tag="sc", name="sc", bufs=2)
                for ki, coff, w, qoff in segs:
                    nc.tensor.matmul(
                        sc[:, coff : coff + w],
                        lhsT=kt[:, b, ts(ki, 128)],
                        rhs=qt[:, b, g, 512 * sq + qoff : 512 * (sq + 1)],
                        start=True,
                        stop=True,
                    )
                return sc

            def emit_act_mask(item, sc):
                segs, is_diag = item[0]
                ncols = max(s[1] + s[2] for s in segs)
                pt = pt_pool.tile([128, SCW], BF16, tag="pt", name="pt")
                nc.scalar.activation(
                    pt[:, 0:ncols],
                    sc[:, 0:ncols],
                    mybir.ActivationFunctionType.Exp,
                    scale=SCALE,
                )
                if is_diag:
                    for _, coff, _, _ in segs:
                        nc.vector.tensor_mul(
                            pt[:, coff : coff + 128], pt[:, coff : coff + 128], mask
                        )
                return pt

            def emit_po_zero():
                # Zero-fill the shared PV accumulator banks with an all-zeros
                # stationary so every element's accumulate bit is set, making
                # the accumulation independent of bank-clear semantics.
                nc.tensor.matmul(
                    po_a[:, :, :], lhsT=zeros, rhs=kt[:, 0, 0:390],
                    start=True, stop=False, skip_group_check=True,
                )
                nc.tensor.matmul(
                    po_b, lhsT=zeros, rhs=kt[:, 0, 0:129],
                    start=True, stop=False, skip_group_check=True,
                )

            def emit_norm_j(j, recips):
                r = small_pool.tile([128, 1], F32, name="recip")
                nc.vector.reciprocal(r, po_ap(j)[:, 128:129])
                recips[j] = r

            def emit_pv_norm(item, pt):
                (segs, is_diag), b, g, sq, first, last = item
                if first:
                    emit_po_zero()
                if not is_diag:
                    for ki, coff, w, qoff in segs:
                        for jq in range(4):
                            nc.tensor.matmul(
                                po_ap(jq),
                                lhsT=pt[:, coff + 128 * jq : coff + 128 * (jq + 1)],
                                rhs=vx[:, b, ki, :],
                                start=False,
                                stop=False,
                                skip_group_check=True,
                            )
                    return
                # diagonal superblock: seg j contributes to po[jq >= j]; po[j]
                # completes at seg j, so normalization interleaves with PV.
                recips = {}
                o_all = oall_pool.tile([128, 4, 128], F32, name="o_all")
                for ki, coff, w, qoff in segs[0:3]:
                    jk = qoff // 128
                    for jq in range(jk, 4):
                        nc.tensor.matmul(
                            po_ap(jq),
                            lhsT=pt[:, coff + 128 * (jq - jk) : coff + 128 * (jq - jk + 1)],
                            rhs=vx[:, b, ki, :],
                            start=False,
                            stop=jq == jk,
                            skip_group_check=True,
                        )
                for j in range(3):  # bank of po_a is final after seg j2
                    emit_norm_j(j, recips)
                ki, coff, w, qoff = segs[3]
                nc.tensor.matmul(
                    po_ap(3),
                    lhsT=pt[:, coff : coff + 128],
                    rhs=vx[:, b, ki, :],
                    start=False,
                    stop=True,
                    skip_group_check=True,
                )
                emit_norm_j(3, recips)
                for j in range(4):
                    nc.vector.tensor_scalar_mul(
                        o_all[:, j, :], po_ap(j)[:, 0:128], recips[j]
                    )
                nc.sync.dma_start(
                    out=o_d[b, 512 * sq : 512 * (sq + 1), g, :].rearrange(
                        "(n p) d -> p n d", p=128
                    ),
                    in_=o_all,
                )

            # 2-group-deep software pipeline across all instances
            work = []  # (gr, b, g, sq, first_of_sq, last_of_sq)
            for inst in range(B * GH):
                b, g = divmod(inst, GH)
                for sq in range(NSB):
                    grs = groups_for_sq(sq)
                    for i, gr in enumerate(grs):
                        work.append((gr, b, g, sq, i == 0, i == len(grs) - 1))

            # PV lags 3 steps and precedes QK in the PE stream, so QK(i)
            # completes well before ACT(i-1) drains and the exp activations
            # run back-to-back.
            n = len(work)
            scs = {}
            pts = {}
            for i in range(n + 3):
                if 3 <= i:
                    emit_pv_norm(work[i - 3], pts.pop(i - 3))
                if i < n:
                    scs[i] = emit_qk(work[i])
                if 1 <= i <= n:
                    pts[i - 1] = emit_act_mask(work[i - 1], scs.pop(i - 1))

    nc.finalize()
    return nc


def make_in_maps(query, key, value):
    query = np.asarray(query, dtype=np.float32)
    key = np.asarray(key, dtype=np.float32)
    value = np.asarray(value, dtype=np.float32)
    in_maps = []
    for c in range(NCORES):
        q_c = query[:, :, GH * c : GH * (c + 1), :]  # [B, S, GH, D]
        k_c = key[:, :, c, :]  # [B, S, D]
        v_c = value[:, :, c, :]  # [B, S, D]
        qt = np.ascontiguousarray(q_c.transpose(3, 0, 2, 1)).astype(BF16_NP)
        kt = np.ascontiguousarray(k_c.transpose(2, 0, 1)).astype(BF16_NP)
        # vx[p, b, blk, 0:128] = V[b, 128*blk+p, :]; vx[..., 128] = 1
        v_blk = v_c.reshape(B, NQB, 128, D)  # [B, blk, p, d]
        vx = np.ones((128, B, NQB, 129), dtype=BF16_NP)
        vx[:, :, :, 0:128] = v_blk.transpose(2, 0, 1, 3).astype(BF16_NP)
        in_maps.append({"qt": qt, "kt": kt, "vx": vx})
    return in_maps


def kernel(query, key, value):
    nc = build_nc()
    res = run_bass_kernel_spmd(
        nc, make_in_maps(query, key, value), core_ids=list(range(NCORES))
    )
    outs = [np.asarray(res.results[c]["out"]) for c in range(NCORES)]
    return np.concatenate(outs, axis=2).astype(np.float32)


if __name__ == "__main__":
    rng = np.random.default_rng(0)
    q = rng.standard_normal((B, S, H, D), dtype=np.float32)
    k = rng.standard_normal((B, S, HK, D), dtype=np.float32)
    v = rng.standard_normal((B, S, HK, D), dtype=np.float32)
    out = kernel(q, k, v)
    print("out", out.shape, out.dtype, float(np.abs(out).max()))


# revision 11
# speedup vs baseline: 1.3270x; 1.3270x over previous
"""Causal GQA attention (prefill) on 8 TRN2 NeuronCores — v2.

Problem: B=2, S=2048, H=32 query heads, Hk=8 kv heads, D=128, f32 I/O.
Sharding: tensor-parallel over heads -- core c gets query heads [4c, 4c+4)
and kv head c. Attention is fully independent per head: no collectives.

v2 vs baseline (222us): the baseline was ScalarE-bound — 320 small exp
ACTIVATEs paid a ~352-cycle fixed cost each (~180us ScalarE busy).
  - Q^T/K^T/V+ones are prepared on the HOST (numpy transpose + bf16
    cast): no PE transposes, no DVE casts/copies, minimal input DMA.
  - Scores S^T[k,q] land in rotating 3-bank f32 PSUM regions holding up
    to 3 key-blocks (or the whole packed ragged diagonal superblock);
    exp runs as ~13 big multi-bank ACTIVATEs per instance instead of 40.
  - PV keeps P^T-stationary matmuls streaming V|ones (free softmax
    denominator in psum column 128); the 4 per-superblock accumulators
    share 2 PSUM banks, zero-initialized by a cheap all-zeros matmul so
    accumulate flags never depend on bank-clear semantics.
  - Emission runs a 2-group-deep software pipeline (PE does QK(g+1),
    QK(g+2) while ScalarE exps group g) and normalization is interleaved
    per 128-query-block so accumulator banks recycle without PE stalls.
"""

import ml_dtypes
import numpy as np

import concourse.bass as bass
import concourse.tile as tile
from concourse import bacc, mybir
from concourse.bass import ts
from concourse.bass_utils import run_bass_kernel_spmd
from concourse.masks import make_upper_triangular

B = 2
S = 2048
H = 32
HK = 8
D = 128
NCORES = 8
GH = H // NCORES  # query heads per core (= group size here)
SCALE = 0.08838834764831845  # 1/sqrt(128)

F32 = mybir.dt.float32
BF16 = mybir.dt.bfloat16
BF16_NP = ml_dtypes.bfloat16

NQB = S // 128  # 16 key blocks of 128
NSB = 4  # query superblocks of 512
SCW = 1536  # score region cols (3 PSUM banks of 512 f32)

# Ragged diagonal superblock: seg j covers queries [128j, 512) of the
# superblock. Column packing avoids crossing 512-col bank boundaries:
# [j0 512 | j1 384 | j3 128 | j2 256] -> contiguous cols [0, 1280).
DIAG_COFF = {0: 0, 1: 512, 2: 1024, 3: 896}


def groups_for_sq(sq):
    """Each group: (segs, is_diag); seg = (ki, col_off, width, q_off)."""
    out = []
    clean = list(range(4 * sq))
    for a in range(0, len(clean), 3):
        chunk = clean[a : a + 3]
        out.append(([(ki, 512 * i, 512, 0) for i, ki in enumerate(chunk)], False))
    d = 4 * sq
    out.append((
        [(d + j, DIAG_COFF[j], 512 - 128 * j, 128 * j) for j in range(4)],
        True,
    ))
    return out


def build_nc() -> bass.Bass:
    nc = bacc.Bacc(
        "TRN2", target_bir_lowering=False, debug=False, num_devices=NCORES
    )
    qt_d = nc.declare_dram_parameter("qt", [128, B, GH, S], BF16, isOutput=False)
    kt_d = nc.declare_dram_parameter("kt", [128, B, S], BF16, isOutput=False)
    vx_d = nc.declare_dram_parameter("vx", [128, B, NQB, 129], BF16, isOutput=False)
    o_d = nc.declare_dram_parameter("out", [B, S, GH, D], F32, isOutput=True)

    with tile.TileContext(nc) as tc:
        with (
            tc.tile_pool(name="consts", bufs=1) as consts,
            tc.tile_pool(name="pt", bufs=6) as pt_pool,
            tc.tile_pool(name="oall", bufs=2) as oall_pool,
            tc.tile_pool(name="stg", bufs=2) as stg_pool,
            tc.tile_pool(name="small", bufs=8) as small_pool,
            tc.tile_pool(name="psum", bufs=1, space="PSUM") as psum_pool,
        ):
            # mask[k, q] = 1 where q >= k (keep), 0 where q < k.
            mask = consts.tile([128, 128], BF16)
            make_upper_triangular(nc, mask, val=1.0, diag=True)
            zeros = consts.tile([128, 128], BF16)
            nc.vector.memset(zeros, 0.0)
            scratch = consts.tile([128, 1], BF16)
            # fire the exp ACT_TABLE_LOAD immediately, overlapping input DMA
            nc.scalar.activation(
                scratch, zeros[:, 0:1], mybir.ActivationFunctionType.Exp
            )

            qt = consts.tile([128, B, GH, S], BF16)
            kt = consts.tile([128, B, S], BF16)
            vx = consts.tile([128, B, NQB, 129], BF16)
            # priority order: the first diagonal group's operands come first
            nc.sync.dma_start(out=kt[:, 0, 0:512], in_=kt_d[:, 0, 0:512])
            nc.sync.dma_start(out=qt[:, 0, 0, 0:512], in_=qt_d[:, 0, 0, 0:512])
            nc.sync.dma_start(out=vx[:, 0, 0:4, :], in_=vx_d[:, 0, 0:4, :])
            nc.sync.dma_start(out=kt[:, 0, 512:S], in_=kt_d[:, 0, 512:S])
            nc.sync.dma_start(out=qt[:, 0, 0, 512:S], in_=qt_d[:, 0, 0, 512:S])
            nc.sync.dma_start(out=vx[:, 0, 4:NQB, :], in_=vx_d[:, 0, 4:NQB, :])
            for g in range(1, GH):
                nc.sync.dma_start(out=qt[:, 0, g, :], in_=qt_d[:, 0, g, :])
            nc.sync.dma_start(out=kt[:, 1, :], in_=kt_d[:, 1, :])
            nc.sync.dma_start(out=vx[:, 1, :, :], in_=vx_d[:, 1, :, :])
            for g in range(GH):
                nc.sync.dma_start(out=qt[:, 1, g, :], in_=qt_d[:, 1, g, :])

            # Persistent per-sq PV accumulators. po_a packs j=0..2 in one
            # bank (130-col stride keeps 8B alignment), po_b holds j=3.
            po_a = psum_pool.tile([128, 3, 130], F32, name="po_a")
            po_b = psum_pool.tile([128, 129], F32, name="po_b")

            # HAM warmup: keep the PE busy while input DMA streams so the
            # clock gate opens (K=8/8) before the first real matmul.
            for _ in range(32):
                nc.tensor.matmul(
                    po_a[:, 0, 0:128], lhsT=zeros, rhs=zeros,
                    start=True, stop=True, skip_group_check=True,
                )

            def po_ap(j):
                return po_a[:, j, 0:129] if j < 3 else po_b

            def emit_qk(item):
                (segs, _), b, g, sq = item[0], item[1], item[2], item[3]
                sc = psum_pool.tile([128, SCW], F32, tag="sc", name="sc", bufs=2)
                for ki, coff, w, qoff in segs:
                    nc.tensor.matmul(
                        sc[:, coff : coff + w],
                        lhsT=kt[:, b, ts(ki, 128)],
                        rhs=qt[:, b, g, 512 * sq + qoff : 512 * (sq + 1)],
                        start=True,
                        stop=True,
                    )
                return sc

            def emit_act_mask(item, sc):
                segs, is_diag = item[0]
                ncols = max(s[1] + s[2] for s in segs)
                pt = pt_pool.tile([128, SCW], BF16, tag="pt", name="pt")
                nc.scalar.activation(
                    pt[:, 0:ncols],
                    sc[:, 0:ncols],
                    mybir.ActivationFunctionType.Exp,
                    scale=SCALE,
                )
                if is_diag:
                    for _, coff, _, _ in segs:
                        nc.vector.tensor_mul(
                            pt[:, coff : coff + 128], pt[:, coff : coff + 128], mask
                        )
                return pt

            def emit_po_zero():
                # Zero-fill the shared PV accumulator banks with an all-zeros
                # stationary so every element's accumulate bit is set, making
                # the accumulation independent of bank-clear semantics.
                nc.tensor.matmul(
                    po_a[:, :, :], lhsT=zeros, rhs=kt[:, 0, 0:390],
                    start=True, stop=False, skip_group_check=True,
                )
                nc.tensor.matmul(
                    po_b, lhsT=zeros, rhs=kt[:, 0, 0:129],
                    start=True, stop=False, skip_group_check=True,
                )

            def emit_evac_norm_store(b, g, sq):
                # Evacuate the PV accumulators to SBUF with two fast copies
                # (releases the po banks for the next superblock), then
                # normalize from SBUF at 2x DVE mode.
                stg_a = stg_pool.tile([128, 3, 130], F32, name="stg_a")
                stg_b = stg_pool.tile([128, 129], F32, name="stg_b")
                nc.vector.tensor_copy(stg_a, po_a)
                nc.vector.tensor_copy(stg_b, po_b)
                rec_a = small_pool.tile([128, 3], F32, name="rec_a")
                rec_b = small_pool.tile([128, 1], F32, name="rec_b")
                nc.vector.reciprocal(rec_a, stg_a[:, :, 128])
                nc.vector.reciprocal(rec_b, stg_b[:, 128:129])
                o_all = oall_pool.tile([128, 4, 128], F32, name="o_all")
                for j in range(3):
                    nc.vector.tensor_scalar_mul(
                        o_all[:, j, :], stg_a[:, j, 0:128], rec_a[:, j : j + 1]
                    )
                nc.vector.tensor_scalar_mul(
                    o_all[:, 3, :], stg_b[:, 0:128], rec_b
                )
                nc.sync.dma_start(
                    out=o_d[b, 512 * sq : 512 * (sq + 1), g, :].rearrange(
                        "(n p) d -> p n d", p=128
                    ),
                    in_=o_all,
                )

            def emit_pv_norm(item, pt):
                (segs, is_diag), b, g, sq, first, last = item
                if first:
                    emit_po_zero()
                if not is_diag:
                    for ki, coff, w, qoff in segs:
                        for jq in range(4):
                            nc.tensor.matmul(
                                po_ap(jq),
                                lhsT=pt[:, coff + 128 * jq : coff + 128 * (jq + 1)],
                                rhs=vx[:, b, ki, :],
                                start=False,
                                stop=False,
                                skip_group_check=True,
                            )
                    return
                # diagonal superblock: seg j contributes to po[jq >= j]
                for ki, coff, w, qoff in segs:
                    jk = qoff // 128
                    for jq in range(jk, 4):
                        nc.tensor.matmul(
                            po_ap(jq),
                            lhsT=pt[:, coff + 128 * (jq - jk) : coff + 128 * (jq - jk + 1)],
                            rhs=vx[:, b, ki, :],
                            start=False,
                            stop=jq == jk,
                            skip_group_check=True,
                        )
                emit_evac_norm_store(b, g, sq)

            # 2-group-deep software pipeline across all instances
            work = []  # (gr, b, g, sq, first_of_sq, last_of_sq)
            for inst in range(B * GH):
                b, g = divmod(inst, GH)
                for sq in range(NSB):
                    grs = groups_for_sq(sq)
                    for i, gr in enumerate(grs):
                        work.append((gr, b, g, sq, i == 0, i == len(grs) - 1))

            # PV lags 3 steps and precedes QK in the PE stream, so QK(i)
            # completes well before ACT(i-1) drains and the exp activations
            # run back-to-back.
            n = len(work)
            scs = {}
            pts = {}
            for i in range(n + 3):
                if 3 <= i:
                    emit_pv_norm(work[i - 3], pts.pop(i - 3))
                if i < n:
                    scs[i] = emit_qk(work[i])
                if 1 <= i <= n:
                    pts[i - 1] = emit_act_mask(work[i - 1], scs.pop(i - 1))

    nc.finalize()
    return nc


def make_in_maps(query, key, value):
    query = np.asarray(query, dtype=np.float32)
    key = np.asarray(key, dtype=np.float32)
    value = np.asarray(value, dtype=np.float32)
    in_maps = []
    for c in range(NCORES):
        q_c = query[:, :, GH * c : GH * (c + 1), :]  # [B, S, GH, D]
        k_c = key[:, :, c, :]  # [B, S, D]
        v_c = value[:, :, c, :]  # [B, S, D]
        qt = np.ascontiguousarray(q_c.transpose(3, 0, 2, 1)).astype(BF16_NP)
        kt = np.ascontiguousarray(k_c.transpose(2, 0, 1)).astype(BF16_NP)
        # vx[p, b, blk, 0:128] = V[b, 128*blk+p, :]; vx[..., 128] = 1
        v_blk = v_c.reshape(B, NQB, 128, D)  # [B, blk, p, d]
        vx = np.ones((128, B, NQB, 129), dtype=BF16_NP)
        vx[:, :, :, 0:128] = v_blk.transpose(2, 0, 1, 3).astype(BF16_NP)
        in_maps.append({"qt": qt, "kt": kt, "vx": vx})
    return in_maps


def kernel(query, key, value):
    nc = build_nc()
    res = run_bass_kernel_spmd(
        nc, make_in_maps(query, key, value), core_ids=list(range(NCORES))
    )
    outs = [np.asarray(res.results[c]["out"]) for c in range(NCORES)]
    return np.concatenate(outs, axis=2).astype(np.float32)


if __name__ == "__main__":
    rng = np.random.default_rng(0)
    q = rng.standard_normal((B, S, H, D), dtype=np.float32)
    k = rng.standard_normal((B, S, HK, D), dtype=np.float32)
    v = rng.standard_normal((B, S, HK, D), dtype=np.float32)
    out = kernel(q, k, v)
    print("out", out.shape, out.dtype, float(np.abs(out).max()))


# revision 12
# speedup vs baseline: 1.3440x; 1.0128x over previous
"""Causal GQA attention (prefill) on 8 TRN2 NeuronCores — v2.

Problem: B=2, S=2048, H=32 query heads, Hk=8 kv heads, D=128, f32 I/O.
Sharding: tensor-parallel over heads -- core c gets query heads [4c, 4c+4)
and kv head c. Attention is fully independent per head: no collectives.

v2 vs baseline (222us): the baseline was ScalarE-bound — 320 small exp
ACTIVATEs paid a ~352-cycle fixed cost each (~180us ScalarE busy).
  - Q^T/K^T/V+ones are prepared on the HOST (numpy transpose + bf16
    cast): no PE transposes, no DVE casts/copies, minimal input DMA.
  - Scores S^T[k,q] land in rotating 3-bank f32 PSUM regions holding up
    to 3 key-blocks (or the whole packed ragged diagonal superblock);
    exp runs as ~13 big multi-bank ACTIVATEs per instance instead of 40.
  - PV keeps P^T-stationary matmuls streaming V|ones (free softmax
    denominator in psum column 128); the 4 per-superblock accumulators
    share 2 PSUM banks, zero-initialized by a cheap all-zeros matmul so
    accumulate flags never depend on bank-clear semantics.
  - Emission runs a 2-group-deep software pipeline (PE does QK(g+1),
    QK(g+2) while ScalarE exps group g) and normalization is interleaved
    per 128-query-block so accumulator banks recycle without PE stalls.
"""

import ml_dtypes
import numpy as np

import concourse.bass as bass
import concourse.tile as tile
from concourse import bacc, mybir
from concourse.bass import ts
from concourse.bass_utils import run_bass_kernel_spmd
from concourse.masks import make_upper_triangular

B = 2
S = 2048
H = 32
HK = 8
D = 128
NCORES = 8
GH = H // NCORES  # query heads per core (= group size here)
SCALE = 0.08838834764831845  # 1/sqrt(128)

F32 = mybir.dt.float32
BF16 = mybir.dt.bfloat16
BF16_NP = ml_dtypes.bfloat16

NQB = S // 128  # 16 key blocks of 128
NSB = 4  # query superblocks of 512
SCW = 1536  # score region cols (3 PSUM banks of 512 f32)

# Ragged diagonal superblock: seg j covers queries [128j, 512) of the
# superblock. Column packing avoids crossing 512-col bank boundaries:
# [j0 512 | j1 384 | j3 128 | j2 256] -> contiguous cols [0, 1280).
DIAG_COFF = {0: 0, 1: 512, 2: 1024, 3: 896}


def groups_for_sq(sq):
    """Each group: (segs, is_diag); seg = (ki, col_off, width, q_off)."""
    out = []
    clean = list(range(4 * sq))
    for a in range(0, len(clean), 3):
        chunk = clean[a : a + 3]
        out.append(([(ki, 512 * i, 512, 0) for i, ki in enumerate(chunk)], False))
    d = 4 * sq
    out.append((
        [(d + j, DIAG_COFF[j], 512 - 128 * j, 128 * j) for j in range(4)],
        True,
    ))
    return out


def build_nc() -> bass.Bass:
    nc = bacc.Bacc(
        "TRN2", target_bir_lowering=False, debug=False, num_devices=NCORES
    )
    qt_d = nc.declare_dram_parameter("qt", [128, B, GH, S], BF16, isOutput=False)
    kt_d = nc.declare_dram_parameter("kt", [128, B, S], BF16, isOutput=False)
    vx_d = nc.declare_dram_parameter("vx", [128, B, NQB, 129], BF16, isOutput=False)
    o_d = nc.declare_dram_parameter("out", [B, S, GH, D], F32, isOutput=True)

    with tile.TileContext(nc) as tc:
        with (
            tc.tile_pool(name="consts", bufs=1) as consts,
            tc.tile_pool(name="pt", bufs=6) as pt_pool,
            tc.tile_pool(name="oall", bufs=2) as oall_pool,
            tc.tile_pool(name="stg", bufs=2) as stg_pool,
            tc.tile_pool(name="small", bufs=8) as small_pool,
            tc.tile_pool(name="psum", bufs=1, space="PSUM") as psum_pool,
        ):
            # mask[k, q] = 1 where q >= k (keep), 0 where q < k.
            mask = consts.tile([128, 128], BF16)
            make_upper_triangular(nc, mask, val=1.0, diag=True)
            zeros = consts.tile([128, 128], BF16)
            nc.vector.memset(zeros, 0.0)
            scratch = consts.tile([128, 1], BF16)
            # fire the exp ACT_TABLE_LOAD immediately, overlapping input DMA
            nc.scalar.activation(
                scratch, zeros[:, 0:1], mybir.ActivationFunctionType.Exp
            )

            qt = consts.tile([128, B, GH, S], BF16)
            kt = consts.tile([128, B, S], BF16)
            vx = consts.tile([128, B, NQB, 129], BF16)
            # priority order: the first diagonal group's operands come first
            nc.sync.dma_start(out=kt[:, 0, 0:512], in_=kt_d[:, 0, 0:512])
            nc.sync.dma_start(out=qt[:, 0, 0, 0:512], in_=qt_d[:, 0, 0, 0:512])
            nc.sync.dma_start(out=vx[:, 0, 0:4, :], in_=vx_d[:, 0, 0:4, :])
            nc.sync.dma_start(out=kt[:, 0, 512:S], in_=kt_d[:, 0, 512:S])
            nc.sync.dma_start(out=qt[:, 0, 0, 512:S], in_=qt_d[:, 0, 0, 512:S])
            nc.sync.dma_start(out=vx[:, 0, 4:NQB, :], in_=vx_d[:, 0, 4:NQB, :])
            for g in range(1, GH):
                nc.sync.dma_start(out=qt[:, 0, g, :], in_=qt_d[:, 0, g, :])
            nc.sync.dma_start(out=kt[:, 1, :], in_=kt_d[:, 1, :])
            nc.sync.dma_start(out=vx[:, 1, :, :], in_=vx_d[:, 1, :, :])
            for g in range(GH):
                nc.sync.dma_start(out=qt[:, 1, g, :], in_=qt_d[:, 1, g, :])

            # Persistent per-sq PV accumulators. po_a packs j=0..2 in one
            # bank (130-col stride keeps 8B alignment), po_b holds j=3.
            po_a = psum_pool.tile([128, 3, 130], F32, name="po_a")
            po_b = psum_pool.tile([128, 129], F32, name="po_b")

            # HAM warmup: keep the PE busy while input DMA streams so the
            # clock gate opens (K=8/8) before the first real matmul.
            for _ in range(56):
                nc.tensor.matmul(
                    po_a[:, 0, 0:128], lhsT=zeros, rhs=zeros,
                    start=True, stop=True, skip_group_check=True,
                )

            def po_ap(j):
                return po_a[:, j, 0:129] if j < 3 else po_b

            def emit_qk(item):
                (segs, _), b, g, sq = item[0], item[1], item[2], item[3]
                sc = psum_pool.tile([128, SCW], F32, tag="sc", name="sc", bufs=2)
                for ki, coff, w, qoff in segs:
                    nc.tensor.matmul(
                        sc[:, coff : coff + w],
                        lhsT=kt[:, b, ts(ki, 128)],
                        rhs=qt[:, b, g, 512 * sq + qoff : 512 * (sq + 1)],
                        start=True,
                        stop=True,
                    )
                return sc

            def emit_act_mask(item, sc):
                segs, is_diag = item[0]
                ncols = max(s[1] + s[2] for s in segs)
                pt = pt_pool.tile([128, SCW], BF16, tag="pt", name="pt")
                nc.scalar.activation(
                    pt[:, 0:ncols],
                    sc[:, 0:ncols],
                    mybir.ActivationFunctionType.Exp,
                    scale=SCALE,
                )
                if is_diag:
                    for _, coff, _, _ in segs:
                        nc.vector.tensor_mul(
                            pt[:, coff : coff + 128], pt[:, coff : coff + 128], mask
                        )
                return pt

            def emit_po_zero():
                # Zero-fill the shared PV accumulator banks with an all-zeros
                # stationary so every element's accumulate bit is set, making
                # the accumulation independent of bank-clear semantics.
                nc.tensor.matmul(
                    po_a[:, :, :], lhsT=zeros, rhs=kt[:, 0, 0:390],
                    start=True, stop=False, skip_group_check=True,
                )
                nc.tensor.matmul(
                    po_b, lhsT=zeros, rhs=kt[:, 0, 0:129],
                    start=True, stop=False, skip_group_check=True,
                )

            def emit_evac_norm_store(b, g, sq):
                # Evacuate the PV accumulators to SBUF with two fast copies
                # (releases the po banks for the next superblock), then
                # normalize from SBUF at 2x DVE mode.
                stg_a = stg_pool.tile([128, 3, 130], F32, name="stg_a")
                stg_b = stg_pool.tile([128, 129], F32, name="stg_b")
                nc.vector.tensor_copy(stg_a, po_a)
                nc.vector.tensor_copy(stg_b, po_b)
                rec_a = small_pool.tile([128, 3], F32, name="rec_a")
                rec_b = small_pool.tile([128, 1], F32, name="rec_b")
                nc.vector.reciprocal(rec_a, stg_a[:, :, 128])
                nc.vector.reciprocal(rec_b, stg_b[:, 128:129])
                o_all = oall_pool.tile([128, 4, 128], F32, name="o_all")
                for j in range(3):
                    nc.vector.tensor_scalar_mul(
                        o_all[:, j, :], stg_a[:, j, 0:128], rec_a[:, j : j + 1]
                    )
                nc.vector.tensor_scalar_mul(
                    o_all[:, 3, :], stg_b[:, 0:128], rec_b
                )
                nc.sync.dma_start(
                    out=o_d[b, 512 * sq : 512 * (sq + 1), g, :].rearrange(
                        "(n p) d -> p n d", p=128
                    ),
                    in_=o_all,
                )

            def emit_pv_norm(item, pt):
                (segs, is_diag), b, g, sq, first, last = item
                if first:
                    emit_po_zero()
                if not is_diag:
                    for ki, coff, w, qoff in segs:
                        for jq in range(4):
                            nc.tensor.matmul(
                                po_ap(jq),
                                lhsT=pt[:, coff + 128 * jq : coff + 128 * (jq + 1)],
                                rhs=vx[:, b, ki, :],
                                start=False,
                                stop=False,
                                skip_group_check=True,
                            )
                    return
                # diagonal superblock: seg j contributes to po[jq >= j]
                for ki, coff, w, qoff in segs:
                    jk = qoff // 128
                    for jq in range(jk, 4):
                        nc.tensor.matmul(
                            po_ap(jq),
                            lhsT=pt[:, coff + 128 * (jq - jk) : coff + 128 * (jq - jk + 1)],
                            rhs=vx[:, b, ki, :],
                            start=False,
                            stop=jq == jk,
                            skip_group_check=True,
                        )
                emit_evac_norm_store(b, g, sq)

            # 2-group-deep software pipeline across all instances
            work = []  # (gr, b, g, sq, first_of_sq, last_of_sq)
            for inst in range(B * GH):
                b, g = divmod(inst, GH)
                for sq in range(NSB):
                    grs = groups_for_sq(sq)
                    for i, gr in enumerate(grs):
                        work.append((gr, b, g, sq, i == 0, i == len(grs) - 1))

            # PV lags 3 steps and precedes QK in the PE stream, so QK(i)
            # completes well before ACT(i-1) drains and the exp activations
            # run back-to-back.
            n = len(work)
            scs = {}
            pts = {}
            for i in range(n + 3):
                if 3 <= i:
                    emit_pv_norm(work[i - 3], pts.pop(i - 3))
                if i < n:
                    scs[i] = emit_qk(work[i])
                if 1 <= i <= n:
                    pts[i - 1] = emit_act_mask(work[i - 1], scs.pop(i - 1))

    nc.finalize()
    return nc


def make_in_maps(query, key, value):
    query = np.asarray(query, dtype=np.float32)
    key = np.asarray(key, dtype=np.float32)
    value = np.asarray(value, dtype=np.float32)
    in_maps = []
    for c in range(NCORES):
        q_c = query[:, :, GH * c : GH * (c + 1), :]  # [B, S, GH, D]
        k_c = key[:, :, c, :]  # [B, S, D]
        v_c = value[:, :, c, :]  # [B, S, D]
        qt = np.ascontiguousarray(q_c.transpose(3, 0, 2, 1)).astype(BF16_NP)
        kt = np.ascontiguousarray(k_c.transpose(2, 0, 1)).astype(BF16_NP)
        # vx[p, b, blk, 0:128] = V[b, 128*blk+p, :]; vx[..., 128] = 1
        v_blk = v_c.reshape(B, NQB, 128, D)  # [B, blk, p, d]
        vx = np.ones((128, B, NQB, 129), dtype=BF16_NP)
        vx[:, :, :, 0:128] = v_blk.transpose(2, 0, 1, 3).astype(BF16_NP)
        in_maps.append({"qt": qt, "kt": kt, "vx": vx})
    return in_maps


def kernel(query, key, value):
    nc = build_nc()
    res = run_bass_kernel_spmd(
        nc, make_in_maps(query, key, value), core_ids=list(range(NCORES))
    )
    outs = [np.asarray(res.results[c]["out"]) for c in range(NCORES)]
    return np.concatenate(outs, axis=2).astype(np.float32)


if __name__ == "__main__":
    rng = np.random.default_rng(0)
    q = rng.standard_normal((B, S, H, D), dtype=np.float32)
    k = rng.standard_normal((B, S, HK, D), dtype=np.float32)
    v = rng.standard_normal((B, S, HK, D), dtype=np.float32)
    out = kernel(q, k, v)
    print("out", out.shape, out.dtype, float(np.abs(out).max()))


# revision 15
# speedup vs baseline: 1.3747x; 1.0229x over previous
"""Causal GQA attention (prefill) on 8 TRN2 NeuronCores — v2.

Problem: B=2, S=2048, H=32 query heads, Hk=8 kv heads, D=128, f32 I/O.
Sharding: tensor-parallel over heads -- core c gets query heads [4c, 4c+4)
and kv head c. Attention is fully independent per head: no collectives.

v2 vs baseline (222us): the baseline was ScalarE-bound — 320 small exp
ACTIVATEs paid a ~352-cycle fixed cost each (~180us ScalarE busy).
  - Q^T/K^T/V+ones are prepared on the HOST (numpy transpose + bf16
    cast): no PE transposes, no DVE casts/copies, minimal input DMA.
  - Scores S^T[k,q] land in rotating 3-bank f32 PSUM regions holding up
    to 3 key-blocks (or the whole packed ragged diagonal superblock);
    exp runs as ~13 big multi-bank ACTIVATEs per instance instead of 40.
  - PV keeps P^T-stationary matmuls streaming V|ones (free softmax
    denominator in psum column 128); the 4 per-superblock accumulators
    share 2 PSUM banks, zero-initialized by a cheap all-zeros matmul so
    accumulate flags never depend on bank-clear semantics.
  - Emission runs a 2-group-deep software pipeline (PE does QK(g+1),
    QK(g+2) while ScalarE exps group g) and normalization is interleaved
    per 128-query-block so accumulator banks recycle without PE stalls.
"""

import ml_dtypes
import numpy as np

import concourse.bass as bass
import concourse.tile as tile
from concourse import bacc, mybir
from concourse.bass import ts
from concourse.bass_utils import run_bass_kernel_spmd
from concourse.masks import make_upper_triangular

B = 2
S = 2048
H = 32
HK = 8
D = 128
NCORES = 8
GH = H // NCORES  # query heads per core (= group size here)
SCALE = 0.08838834764831845  # 1/sqrt(128)

F32 = mybir.dt.float32
BF16 = mybir.dt.bfloat16
BF16_NP = ml_dtypes.bfloat16

NQB = S // 128  # 16 key blocks of 128
NSB = 4  # query superblocks of 512
SCW = 1536  # score region cols (3 PSUM banks of 512 f32)

# Ragged diagonal superblock: seg j covers queries [128j, 512) of the
# superblock. Column packing avoids crossing 512-col bank boundaries:
# [j0 512 | j1 384 | j3 128 | j2 256] -> contiguous cols [0, 1280).
DIAG_COFF = {0: 0, 1: 512, 2: 1024, 3: 896}


def groups_for_sq(sq):
    """Each group: (segs, is_diag); seg = (ki, col_off, width, q_off)."""
    out = []
    clean = list(range(4 * sq))
    for a in range(0, len(clean), 3):
        chunk = clean[a : a + 3]
        out.append(([(ki, 512 * i, 512, 0) for i, ki in enumerate(chunk)], False))
    d = 4 * sq
    out.append((
        [(d + j, DIAG_COFF[j], 512 - 128 * j, 128 * j) for j in range(4)],
        True,
    ))
    return out


def build_nc() -> bass.Bass:
    nc = bacc.Bacc(
        "TRN2", target_bir_lowering=False, debug=False, num_devices=NCORES
    )
    qt_d = nc.declare_dram_parameter("qt", [128, B, GH, S], BF16, isOutput=False)
    kt_d = nc.declare_dram_parameter("kt", [128, B, S], BF16, isOutput=False)
    vx_d = nc.declare_dram_parameter("vx", [128, B, NQB, 129], BF16, isOutput=False)
    o_d = nc.declare_dram_parameter("out", [B, S, GH, D], F32, isOutput=True)

    with tile.TileContext(nc) as tc:
        with (
            tc.tile_pool(name="consts", bufs=1) as consts,
            tc.tile_pool(name="pt", bufs=6) as pt_pool,
            tc.tile_pool(name="oall", bufs=2) as oall_pool,
            tc.tile_pool(name="stg", bufs=2) as stg_pool,
            tc.tile_pool(name="small", bufs=8) as small_pool,
            tc.tile_pool(name="psum", bufs=1, space="PSUM") as psum_pool,
        ):
            # mask[k, q] = 1 where q >= k (keep), 0 where q < k.
            mask = consts.tile([128, 128], BF16)
            make_upper_triangular(nc, mask, val=1.0, diag=True)
            zeros = consts.tile([128, 128], BF16)
            nc.vector.memset(zeros, 0.0)
            scratch = consts.tile([128, 1], BF16)
            # fire the exp ACT_TABLE_LOAD immediately, overlapping input DMA
            nc.scalar.activation(
                scratch, zeros[:, 0:1], mybir.ActivationFunctionType.Exp
            )

            qt = consts.tile([128, B, GH, S], BF16)
            kt = consts.tile([128, B, S], BF16)
            vx = consts.tile([128, B, NQB, 129], BF16)
            # priority order: the first diagonal group's operands come first
            nc.sync.dma_start(out=kt[:, 0, 0:512], in_=kt_d[:, 0, 0:512])
            nc.sync.dma_start(out=qt[:, 0, 0, 0:512], in_=qt_d[:, 0, 0, 0:512])
            nc.sync.dma_start(out=vx[:, 0, 0:4, :], in_=vx_d[:, 0, 0:4, :])
            nc.sync.dma_start(out=kt[:, 0, 512:S], in_=kt_d[:, 0, 512:S])
            nc.sync.dma_start(out=qt[:, 0, 0, 512:S], in_=qt_d[:, 0, 0, 512:S])
            nc.sync.dma_start(out=vx[:, 0, 4:NQB, :], in_=vx_d[:, 0, 4:NQB, :])
            for g in range(1, GH):
                nc.sync.dma_start(out=qt[:, 0, g, :], in_=qt_d[:, 0, g, :])
            nc.sync.dma_start(out=kt[:, 1, :], in_=kt_d[:, 1, :])
            nc.sync.dma_start(out=vx[:, 1, :, :], in_=vx_d[:, 1, :, :])
            for g in range(GH):
                nc.sync.dma_start(out=qt[:, 1, g, :], in_=qt_d[:, 1, g, :])

            # Persistent per-sq PV accumulators. po_a packs j=0..2 in one
            # bank (130-col stride keeps 8B alignment), po_b holds j=3.
            po_a = psum_pool.tile([128, 3, 130], F32, name="po_a")
            po_b = psum_pool.tile([128, 129], F32, name="po_b")

            # HAM warmup: keep the PE busy while input DMA streams so the
            # clock gate opens (K=8/8) before the first real matmul.
            for _ in range(56):
                nc.tensor.matmul(
                    po_a[:, 0, 0:128], lhsT=zeros, rhs=zeros,
                    start=True, stop=True, skip_group_check=True,
                )

            def po_ap(j):
                return po_a[:, j, 0:129] if j < 3 else po_b

            def emit_qk(item):
                (segs, _), b, g, sq = item[0], item[1], item[2], item[3]
                sc = psum_pool.tile([128, SCW], F32, tag="sc", name="sc", bufs=2)
                for ki, coff, w, qoff in segs:
                    nc.tensor.matmul(
                        sc[:, coff : coff + w],
                        lhsT=kt[:, b, ts(ki, 128)],
                        rhs=qt[:, b, g, 512 * sq + qoff : 512 * (sq + 1)],
                        start=True,
                        stop=True,
                    )
                return sc

            def emit_act_mask(item, sc):
                segs, is_diag = item[0]
                ncols = max(s[1] + s[2] for s in segs)
                pt = pt_pool.tile([128, SCW], BF16, tag="pt", name="pt")
                nc.scalar.activation(
                    pt[:, 0:ncols],
                    sc[:, 0:ncols],
                    mybir.ActivationFunctionType.Exp,
                    scale=SCALE,
                )
                if is_diag:
                    for _, coff, _, _ in segs:
                        nc.vector.tensor_mul(
                            pt[:, coff : coff + 128], pt[:, coff : coff + 128], mask
                        )
                return pt

            def emit_po_zero():
                # Zero-fill the shared PV accumulator banks with an all-zeros
                # stationary so every element's accumulate bit is set, making
                # the accumulation independent of bank-clear semantics.
                nc.tensor.matmul(
                    po_a[:, :, :], lhsT=zeros, rhs=kt[:, 0, 0:390],
                    start=True, stop=False, skip_group_check=True,
                )
                nc.tensor.matmul(
                    po_b, lhsT=zeros, rhs=kt[:, 0, 0:129],
                    start=True, stop=False, skip_group_check=True,
                )

            def emit_evac_norm_store(b, g, sq):
                # Evacuate the PV accumulators to SBUF with two fast copies
                # (releases the po banks for the next superblock), then
                # normalize from SBUF at 2x DVE mode.
                stg_a = stg_pool.tile([128, 3, 130], F32, name="stg_a")
                stg_b = stg_pool.tile([128, 129], F32, name="stg_b")
                nc.vector.tensor_copy(stg_a, po_a)
                nc.vector.tensor_copy(stg_b, po_b)
                rec_a = small_pool.tile([128, 3], F32, name="rec_a")
                rec_b = small_pool.tile([128, 1], F32, name="rec_b")
                nc.vector.reciprocal(rec_a, stg_a[:, :, 128])
                nc.vector.reciprocal(rec_b, stg_b[:, 128:129])
                o_all = oall_pool.tile([128, 4, 128], F32, name="o_all")
                for j in range(3):
                    nc.vector.tensor_scalar_mul(
                        o_all[:, j, :], stg_a[:, j, 0:128], rec_a[:, j : j + 1]
                    )
                nc.vector.tensor_scalar_mul(
                    o_all[:, 3, :], stg_b[:, 0:128], rec_b
                )
                nc.sync.dma_start(
                    out=o_d[b, 512 * sq : 512 * (sq + 1), g, :].rearrange(
                        "(n p) d -> p n d", p=128
                    ),
                    in_=o_all,
                )

            def emit_pv_norm(item, pt):
                (segs, is_diag), b, g, sq, first, last = item
                if first:
                    emit_po_zero()
                if not is_diag:
                    for ki, coff, w, qoff in segs:
                        for jq in range(4):
                            nc.tensor.matmul(
                                po_ap(jq),
                                lhsT=pt[:, coff + 128 * jq : coff + 128 * (jq + 1)],
                                rhs=vx[:, b, ki, :],
                                start=False,
                                stop=False,
                                skip_group_check=True,
                            )
                    return
                # diagonal superblock: seg j contributes to po[jq >= j]
                for ki, coff, w, qoff in segs:
                    jk = qoff // 128
                    for jq in range(jk, 4):
                        nc.tensor.matmul(
                            po_ap(jq),
                            lhsT=pt[:, coff + 128 * (jq - jk) : coff + 128 * (jq - jk + 1)],
                            rhs=vx[:, b, ki, :],
                            start=False,
                            stop=jq == jk,
                            skip_group_check=True,
                        )
                emit_evac_norm_store(b, g, sq)

            # 2-group-deep software pipeline across all instances
            work = []  # (gr, b, g, sq, first_of_sq, last_of_sq)
            for inst in range(B * GH):
                b, g = divmod(inst, GH)
                for sq in range(NSB):
                    grs = groups_for_sq(sq)
                    for i, gr in enumerate(grs):
                        work.append((gr, b, g, sq, i == 0, i == len(grs) - 1))

            # PV lags 3 steps and precedes QK in the PE stream, so QK(i)
            # completes well before ACT(i-1) drains and the exp activations
            # run back-to-back. Exception: when the lagged PV item opens or
            # closes a superblock it stalls on the po-accumulator turnaround
            # (zero-fill waits for the SBUF evacuation), so QK goes first to
            # keep the exp stream fed through the turnaround.
            n = len(work)
            scs = {}
            pts = {}
            for i in range(n + 3):
                pv_item = work[i - 3] if i >= 3 else None
                qk_first = pv_item is not None and (pv_item[4] or pv_item[5])
                if qk_first and i < n:
                    scs[i] = emit_qk(work[i])
                if pv_item is not None:
                    emit_pv_norm(pv_item, pts.pop(i - 3))
                if not qk_first and i < n:
                    scs[i] = emit_qk(work[i])
                if 1 <= i <= n:
                    pts[i - 1] = emit_act_mask(work[i - 1], scs.pop(i - 1))

    nc.finalize()
    return nc


def make_in_maps(query, key, value):
    query = np.asarray(query, dtype=np.float32)
    key = np.asarray(key, dtype=np.float32)
    value = np.asarray(value, dtype=np.float32)
    in_maps = []
    for c in range(NCORES):
        q_c = query[:, :, GH * c : GH * (c + 1), :]  # [B, S, GH, D]
        k_c = key[:, :, c, :]  # [B, S, D]
        v_c = value[:, :, c, :]  # [B, S, D]
        qt = np.ascontiguousarray(q_c.transpose(3, 0, 2, 1)).astype(BF16_NP)
        kt = np.ascontiguousarray(k_c.transpose(2, 0, 1)).astype(BF16_NP)
        # vx[p, b, blk, 0:128] = V[b, 128*blk+p, :]; vx[..., 128] = 1
        v_blk = v_c.reshape(B, NQB, 128, D)  # [B, blk, p, d]
        vx = np.ones((128, B, NQB, 129), dtype=BF16_NP)
        vx[:, :, :, 0:128] = v_blk.transpose(2, 0, 1, 3).astype(BF16_NP)
        in_maps.append({"qt": qt, "kt": kt, "vx": vx})
    return in_maps


def kernel(query, key, value):
    nc = build_nc()
    res = run_bass_kernel_spmd(
        nc, make_in_maps(query, key, value), core_ids=list(range(NCORES))
    )
    outs = [np.asarray(res.results[c]["out"]) for c in range(NCORES)]
    return np.concatenate(outs, axis=2).astype(np.float32)


if __name__ == "__main__":
    rng = np.random.default_rng(0)
    q = rng.standard_normal((B, S, H, D), dtype=np.float32)
    k = rng.standard_normal((B, S, HK, D), dtype=np.float32)
    v = rng.standard_normal((B, S, HK, D), dtype=np.float32)
    out = kernel(q, k, v)
    print("out", out.shape, out.dtype, float(np.abs(out).max()))
